# revision 1
# baseline (speedup 1.0000x reference)
"""DifferentiableHPWL on 8 trn2 NeuronCores.

Strategy (sharded by nets, hint-compliant):
  - Host: cast int64 index tensors to int32, bucket nets by pin-count,
    shard nets across 8 cores, compose slot->macro = pin_to_macro[net_to_pin]
    (index metadata only), lay out per-core slot tables [128, slots].
  - Device (per core): build the per-macro record table T2[v] =
    (x[8b], y[8b], c[8b], s[8b]) from positions + rotation_onehot
    (c = oh0-oh2, s = oh1-oh3 computed on device), then per chunk of nets:
    indirect-DMA gather pin offsets (8B/slot) + macro records (128B/slot),
    compute rotated pin positions px,py for all 8 batches, per-net
    softmax-max/min (logsumexp with exact max/min shift), weighted
    accumulation into per-partition per-batch partials [128, 8].
  - Host: sum partials over partitions and cores -> (8,) float32.
"""

import numpy as np

import concourse.bass as bass
import concourse.mybir as mybir
from concourse.tile import TileContext
from concourse import bass_utils

F32 = mybir.dt.float32
I32 = mybir.dt.int32
AX = mybir.AxisListType
ALU = mybir.AluOpType
ACT = mybir.ActivationFunctionType

GAMMA = 10.0
N_CORES = 8
P = 128  # partitions


def _patch_tile_drain():
    """This walrus lowers InstDrain to a TPB_CTRL form with too few sync-wait
    slots; hoist the final drain's waits onto single-wait nops instead."""
    from concourse.vector_clock import ScopedClock

    if getattr(TileContext, "_drain_patched", False):
        return

    def _drain_and_barrier(self, tick_clock, wait_clock):
        nc = self.nc
        carrier = nc.sync.nop(nofuse=True, hint="drain_wait_carrier")
        wait_clock.add_sem_waits(
            carrier.ins, ScopedClock({None: tick_clock.global_clock})
        )
        waits = list(carrier.ins.sync_info.on_wait) if carrier.ins.sync_info else []
        if len(waits) > 1:
            carrier.ins.sync_info = mybir.SyncInfo(on_wait=[waits[0]], on_update=[])
            for w in waits[1:]:
                n2 = nc.sync.nop(nofuse=True, hint="drain_wait_extra")
                n2.ins.sync_info = mybir.SyncInfo(on_wait=[w], on_update=[])
        nc.sync.drain()
        nc.all_engine_barrier()
        popped = nc._tile_sem_poison_stack.pop()
        assert popped is self._sem_poison
        nc.clear_and_free_semaphores(list(self.sems.allocated().values()))
        nc.all_engine_barrier()

    TileContext._drain_and_barrier = _drain_and_barrier
    TileContext._drain_patched = True


def _split_excess_waits(nc, dma_limit=1, other_limit=1):
    """walrus here rejects DMA instructions with >1 sync wait (and drains with
    >1). Hoist excess waits onto same-engine NoOp carriers inserted before the
    instruction — the sequencer executes carrier waits first, preserving
    semantics."""
    ctr = 0
    for f in nc.m.functions:
        for bb in f.blocks:
            out = []
            changed = False
            for inst in bb.instructions:
                si = inst.sync_info
                waits = list(si.on_wait) if si and si.on_wait else []
                if isinstance(inst, (mybir.InstDMACopy, mybir.InstDrain)):
                    limit = dma_limit
                else:
                    limit = other_limit
                if len(waits) > limit:
                    keep = waits[len(waits) - limit:]
                    for w in waits[: len(waits) - limit]:
                        nop = mybir.InstNoOp(name=f"waitsplit-{ctr}")
                        ctr += 1
                        nop.engine = inst.engine
                        nop.sync_info = mybir.SyncInfo(on_wait=[w], on_update=[])
                        nc.register_instruction(nop, overwrite=True)
                        out.append(nop)
                    inst.sync_info = mybir.SyncInfo(
                        on_wait=keep,
                        on_update=list(si.on_update) if si.on_update else [],
                    )
                    changed = True
                out.append(inst)
            if changed:
                bb.instructions = out
    return ctr


def build_program(vpad, ppad, chunk_plan, tot_slot, tot_g):
    """Build the SPMD Bass program.

    vpad: padded macro count (multiple of 128); ppad: padded pin count.
    chunk_plan: list of (k, g, slot_off, g_off) chunks.
    tot_slot: total slots per partition; tot_g: total net-groups/partition.
    """
    _patch_tile_drain()
    nc = bass.Bass("TRN2", target_bir_lowering=False, debug=False,
                   num_swdge_queues=4)

    t1 = nc.dram_tensor("t1", [ppad, 2], F32, kind="ExternalInput")
    posxy = nc.dram_tensor("posxy", [vpad, 16], F32, kind="ExternalInput")
    oh = nc.dram_tensor("oh", [vpad, 32], F32, kind="ExternalInput")
    idx_all = nc.dram_tensor("idx_all", [P, tot_slot], I32, kind="ExternalInput")
    mac_all = nc.dram_tensor("mac_all", [P, tot_slot], I32, kind="ExternalInput")
    w_all = nc.dram_tensor("w_all", [P, tot_g], F32, kind="ExternalInput")
    out = nc.dram_tensor("acc", [P, 8], F32, kind="ExternalOutput")

    nt = vpad // P  # macro tiles

    with TileContext(nc) as tc:
        with (
            tc.tile_pool(name="dram", bufs=1, space="DRAM") as dpool,
            tc.tile_pool(name="persist", bufs=1) as pp,
            tc.tile_pool(name="work", bufs=2) as wp,
        ):
            # ---- build T2 [vpad, 32] in DRAM ----
            t2 = dpool.tile([vpad, 32], F32)
            with tc.tile_pool(name="build", bufs=1) as bp:
                t2img = bp.tile([P, nt * 32], F32)
                t2r = t2img.rearrange("p (t c) -> p t c", t=nt)
                # positions into fields 0:16 via DVE so the T2 writeback DMA
                # has a single (DVE) wait dependency — this walrus rejects
                # DMA instructions carrying 3+ sync waits.
                posxy_t = bp.tile([P, nt * 16], F32)
                nc.sync.dma_start(
                    posxy_t.rearrange("p (t f) -> p t f", t=nt),
                    posxy.ap().rearrange("(t p) f -> p t f", p=P),
                )
                nc.vector.tensor_copy(
                    t2r[:, :, 0:16],
                    posxy_t.rearrange("p (t f) -> p t f", t=nt),
                )
                oh_t = bp.tile([P, nt * 32], F32)
                nc.sync.dma_start(
                    oh_t.rearrange("p (t f) -> p t f", t=nt),
                    oh.ap().rearrange("(t p) f -> p t f", p=P),
                )
                ohr = oh_t.rearrange("p (t b f) -> p t b f", t=nt, f=4)
                # c = oh0 - oh2 -> fields 16:24 ; s = oh1 - oh3 -> fields 24:32
                nc.vector.tensor_tensor(
                    out=t2r[:, :, 16:24], in0=ohr[:, :, :, 0], in1=ohr[:, :, :, 2],
                    op=ALU.subtract,
                )
                nc.vector.tensor_tensor(
                    out=t2r[:, :, 24:32], in0=ohr[:, :, :, 1], in1=ohr[:, :, :, 3],
                    op=ALU.subtract,
                )
                nc.sync.dma_start(
                    t2[:].rearrange("(t p) c -> p t c", p=P), t2r
                )

            # ---- persistent loads ----
            idx_t = pp.tile([P, tot_slot], I32)
            nc.sync.dma_start(idx_t[:], idx_all.ap())
            mac_t = pp.tile([P, tot_slot], I32)
            nc.sync.dma_start(mac_t[:], mac_all.ap())
            w_t = pp.tile([P, tot_g], F32)
            nc.sync.dma_start(w_t[:], w_all.ap())
            acc = pp.tile([P, 8], F32)
            nc.vector.memset(acc[:], 0.0)

            # ---- chunk loop ----
            for (k, g, slot_off, g_off) in chunk_plan:
                cs = g * k  # slots per partition this chunk
                # this walrus supports only ONE dynamic offset per partition
                # per indirect DMA: issue one instruction per slot column
                # (128 gathered rows each), round-robined over 4 SWDGE queues.
                rec1 = wp.tile([P, cs * 2], F32, tag="rec1")
                rec2 = wp.tile([P, cs * 32], F32, tag="rec2")
                # Interleave the tiny (8B/row) T1 reads with the large
                # (128B/row) T2 reads 1:1 so the DMA engines always have
                # burst traffic in flight while the latency-bound T1
                # transactions drain; the queue round-robin then puts T1 on
                # queues 0/2 and T2 on 1/3.
                for j in range(cs):
                    nc.gpsimd.indirect_dma_start(
                        out=rec1[:, 2 * j:2 * j + 2], out_offset=None,
                        in_=t1.ap(),
                        in_offset=bass.IndirectOffsetOnAxis(
                            ap=idx_t[:, slot_off + j:slot_off + j + 1], axis=0),
                    )
                    nc.gpsimd.indirect_dma_start(
                        out=rec2[:, 32 * j:32 * j + 32], out_offset=None,
                        in_=t2[:],
                        in_offset=bass.IndirectOffsetOnAxis(
                            ap=mac_t[:, slot_off + j:slot_off + j + 1], axis=0),
                    )

                r5 = rec2.rearrange("p (g j c) -> p g j c", g=g, j=k)
                Xv = r5[:, :, :, 0:8]
                Yv = r5[:, :, :, 8:16]
                Cv = r5[:, :, :, 16:24]
                Sv = r5[:, :, :, 24:32]
                r1f = rec1.rearrange("p (g j f) -> p g j f", g=g, j=k)
                oxv = r1f[:, :, :, 0:1].to_broadcast([P, g, k, 8])
                oyv = r1f[:, :, :, 1:2].to_broadcast([P, g, k, 8])

                # pv layout [p, (g b c j)] -> j innermost per channel
                pv = wp.tile([P, g * 16 * k], F32, tag="pv")
                pvr = pv.rearrange("p (g b c j) -> p g b c j", g=g, b=8, c=2, j=k)
                pxo = pvr[:, :, :, 0, :].transpose([0, 1, 3, 2])
                pyo = pvr[:, :, :, 1, :].transpose([0, 1, 3, 2])

                ta = wp.tile([P, cs * 8], F32, tag="ta")
                tar = ta.rearrange("p (g j b) -> p g j b", g=g, j=k)
                tb = wp.tile([P, cs * 8], F32, tag="tb")
                tbr = tb.rearrange("p (g j b) -> p g j b", g=g, j=k)

                nc.vector.tensor_tensor(out=tar, in0=Cv, in1=oxv, op=ALU.mult)
                nc.vector.tensor_tensor(out=tbr, in0=Sv, in1=oyv, op=ALU.mult)
                nc.vector.tensor_tensor(out=tar, in0=tar, in1=Xv, op=ALU.add)
                nc.vector.tensor_tensor(out=pxo, in0=tar, in1=tbr, op=ALU.subtract)
                nc.vector.tensor_tensor(out=tar, in0=Sv, in1=oxv, op=ALU.mult)
                nc.vector.tensor_tensor(out=tbr, in0=Cv, in1=oyv, op=ALU.mult)
                nc.vector.tensor_tensor(out=tar, in0=tar, in1=Yv, op=ALU.add)
                nc.vector.tensor_tensor(out=pyo, in0=tar, in1=tbr, op=ALU.add)

                nch = g * 16
                pvs = pv.rearrange("p (s j) -> p s j", j=k)
                Mx = wp.tile([P, nch], F32, tag="Mx")
                mn = wp.tile([P, nch], F32, tag="mn")
                nc.vector.tensor_reduce(out=Mx[:], in_=pvs, axis=AX.X, op=ALU.max)
                nc.vector.tensor_reduce(out=mn[:], in_=pvs, axis=AX.X, op=ALU.min)

                d = wp.tile([P, nch * k], F32, tag="d")
                dr = d.rearrange("p (s j) -> p s j", j=k)
                e = wp.tile([P, nch * k], F32, tag="e")
                er = e.rearrange("p (s j) -> p s j", j=k)
                Sx = wp.tile([P, nch], F32, tag="Sx")
                Sn = wp.tile([P, nch], F32, tag="Sn")
                Mb = Mx.unsqueeze(2).to_broadcast([P, nch, k])
                mb = mn.unsqueeze(2).to_broadcast([P, nch, k])

                nc.vector.tensor_tensor(out=dr, in0=pvs, in1=Mb, op=ALU.subtract)
                nc.scalar.activation(out=e[:], in_=d[:], func=ACT.Exp, scale=GAMMA)
                nc.vector.tensor_reduce(out=Sx[:], in_=er, axis=AX.X, op=ALU.add)
                nc.vector.tensor_tensor(out=dr, in0=pvs, in1=mb, op=ALU.subtract)
                nc.scalar.activation(out=e[:], in_=d[:], func=ACT.Exp, scale=-GAMMA)
                nc.vector.tensor_reduce(out=Sn[:], in_=er, axis=AX.X, op=ALU.add)

                lnx = wp.tile([P, nch], F32, tag="lnx")
                lnn = wp.tile([P, nch], F32, tag="lnn")
                nc.scalar.activation(out=lnx[:], in_=Sx[:], func=ACT.Ln)
                nc.scalar.activation(out=lnn[:], in_=Sn[:], func=ACT.Ln)
                wch = wp.tile([P, nch], F32, tag="wch")
                nc.vector.tensor_tensor(out=wch[:], in0=Mx[:], in1=mn[:], op=ALU.subtract)
                nc.vector.tensor_tensor(out=lnx[:], in0=lnx[:], in1=lnn[:], op=ALU.add)
                nc.scalar.activation(out=lnx[:], in_=lnx[:], func=ACT.Copy,
                                     scale=1.0 / GAMMA)
                nc.vector.tensor_tensor(out=wch[:], in0=wch[:], in1=lnx[:], op=ALU.add)

                wnb = wp.tile([P, g * 8], F32, tag="wnb")
                nc.vector.tensor_reduce(
                    out=wnb[:], in_=wch.rearrange("p (s c) -> p s c", c=2),
                    axis=AX.X, op=ALU.add,
                )
                wbr = w_t[:, g_off:g_off + g].unsqueeze(2).to_broadcast([P, g, 8])
                wnbr = wnb.rearrange("p (g b) -> p g b", g=g)
                nc.vector.tensor_tensor(out=wnbr, in0=wnbr, in1=wbr, op=ALU.mult)
                # reduce over g then accumulate
                part = wp.tile([P, 8], F32, tag="part")
                nc.vector.tensor_reduce(
                    out=part[:], in_=wnbr.transpose([0, 2, 1]), axis=AX.X, op=ALU.add,
                )
                nc.vector.tensor_tensor(out=acc[:], in0=acc[:], in1=part[:], op=ALU.add)

            nc.sync.dma_start(out.ap(), acc[:])
    _split_excess_waits(nc)
    # Post-scheduling: spread Pool indirect DMAs over the 4 SWDGE queues so
    # all Q7 descriptor-gen queues work in parallel. Safe post-Tile: every
    # DMA carries its own completion sem (FIFO-dominance elision disabled).
    qctr = 0
    for f in nc.m.functions:
        for bb in f.blocks:
            for inst in bb.instructions:
                if isinstance(inst, mybir.InstDMACopy) and \
                        inst.queue == "qPoolDynamic":
                    q = qctr % 4
                    qctr += 1
                    if q:
                        inst.queue = f"qPoolDynamic{q}"
    return nc


def prep_host(positions, pin_offsets, rotation_onehot, net_weights,
              net_to_pin, pin_to_macro):
    """Host-side sharding/layout. Returns (meta, in_maps)."""
    B, V, _ = positions.shape
    Pn = pin_offsets.shape[0]
    N, M = net_to_pin.shape

    vpad = ((V + 1 + P - 1) // P) * P  # +1 pad macro row
    ppad = Pn + 1                      # +1 pad pin row
    pad_mac = V
    pad_pin = Pn

    n2p = net_to_pin.astype(np.int32)
    p2m = np.concatenate(
        [pin_to_macro.astype(np.int32), np.array([pad_mac], np.int32)]
    )

    # replicated tables
    t1 = np.zeros((ppad, 2), np.float32)
    t1[:Pn] = pin_offsets
    posxy = np.zeros((vpad, 16), np.float32)
    posxy[:V, 0:8] = positions[:, :, 0].T
    posxy[:V, 8:16] = positions[:, :, 1].T
    oh = np.zeros((vpad, 32), np.float32)
    oh[:V] = rotation_onehot.transpose(1, 0, 2).reshape(V, 4 * B)

    lengths = (n2p >= 0).sum(axis=1)

    # shard nets contiguously
    per = (N + N_CORES - 1) // N_CORES
    shards = [(c * per, min((c + 1) * per, N)) for c in range(N_CORES)]

    # bucket counts per core -> global G_k
    ks = range(1, M + 1)
    counts = np.zeros((N_CORES, M + 1), np.int64)
    for c, (a, b) in enumerate(shards):
        cnt = np.bincount(lengths[a:b], minlength=M + 1)
        counts[c] = cnt
    gk = {k: int(-(-counts[:, k].max() // P)) for k in ks if counts[:, k].max() > 0}

    # chunk plan: split each bucket's G into chunks with cs*32*4B <= 16KB/part
    chunk_plan = []
    slot_off = 0
    g_off = 0
    bucket_offs = {}
    for k in sorted(gk):
        g_total = gk[k]
        gmax = max(1, 128 // k)
        bucket_offs[k] = (slot_off, g_off)
        g_done = 0
        while g_done < g_total:
            g = min(gmax, g_total - g_done)
            chunk_plan.append((k, g, slot_off, g_off))
            slot_off += g * k
            g_off += g
            g_done += g
    tot_slot = slot_off
    tot_g = g_off

    # per-core slot tables
    in_maps = []
    for c, (a, b) in enumerate(shards):
        idx_all = np.full((P, tot_slot), pad_pin, np.int32)
        mac_all = np.full((P, tot_slot), pad_mac, np.int32)
        w_all = np.zeros((P, tot_g), np.float32)
        ln = lengths[a:b]
        for k in sorted(gk):
            so, go = bucket_offs[k]
            sel = np.nonzero(ln == k)[0]
            nk = len(sel)
            if nk == 0:
                continue
            gkk = gk[k]
            ids = n2p[a:b][sel][:, :k]               # (nk, k) valid prefix
            w = net_weights[a:b][sel].astype(np.float32)
            idsp = np.full((gkk * P, k), pad_pin, np.int32)
            idsp[:nk] = ids
            wp_ = np.zeros((gkk * P,), np.float32)
            wp_[:nk] = w
            # net r -> (g=r//P, p=r%P)
            idx_all[:, so:so + gkk * k] = (
                idsp.reshape(gkk, P, k).transpose(1, 0, 2).reshape(P, gkk * k)
            )
            mac_all[:, so:so + gkk * k] = p2m[
                idx_all[:, so:so + gkk * k]
            ]
            w_all[:, go:go + gkk] = wp_.reshape(gkk, P).T
        in_maps.append({
            "t1": t1, "posxy": posxy, "oh": oh,
            "idx_all": idx_all, "mac_all": mac_all, "w_all": w_all,
        })

    meta = (vpad, ppad, tuple(chunk_plan), tot_slot, tot_g)
    return meta, in_maps


_prog_cache = {}


def kernel(**inputs):
    meta, in_maps = prep_host(
        np.asarray(inputs["positions"]),
        np.asarray(inputs["pin_offsets"]),
        np.asarray(inputs["rotation_onehot"]),
        np.asarray(inputs["net_weights"]),
        np.asarray(inputs["net_to_pin"]),
        np.asarray(inputs["pin_to_macro"]),
    )
    if meta not in _prog_cache:
        _prog_cache[meta] = build_program(*meta)
    nc = _prog_cache[meta]
    res = bass_utils.run_bass_kernel_spmd(nc, in_maps, core_ids=list(range(N_CORES)))
    total = np.zeros(8, np.float64)
    for r in res.results:
        total += r["acc"].astype(np.float64).sum(axis=0)
    return total.astype(np.float32)



# revision 28
# speedup vs baseline: 2.3866x; 2.3866x over previous
"""DifferentiableHPWL on 8 trn2 NeuronCores — dma_gather version.

Strategy (sharded by nets per the hint):
  - Host (metadata/layout only): bucket nets by pin count, shard nets
    across 8 cores, compose per-slot int16 gather-index tables for
    InstDMAGatherAnt, one-hot pin-extract masks, and pack pin_offsets
    into a 256B-strided fp16 table (16 pins/row).
  - Device per core:
      * build macro record table T2[m] = gamma*(x[8],y[8],c[8],s[8]) fp16
        (64B payload / 256B stride) from positions + rotation_onehot
      * per compute chunk of <=120 slot columns: 8-column (1024-index,
        the SWDGE ring cap) dma_gather sub-instructions for macro
        records (int16 macro ids) and pin-offset blocks (idx =
        pin//16 - 32768 against a base-shifted ap so signed int16
        covers all 50000 rows), then DVE: mask-extract the pin pair,
        rotate, per-net logsumexp max/min (gamma prefolded), weighted
        accumulate into acc[128, 8].
  - Host: sum partials over partitions and cores.

The SWDGE ucode trims TRAILING negative indices per instruction, so the
host guarantees each pin sub-gather's final position (partition 127 of
its last column) holds a pin >= 16*32768: pad nets use pin 524288 and
real nets are swapped/pin-reordered per group as needed (P(impossible)
is cryptographically small; asserted).
"""

import numpy as np

import concourse.bass as bass
import concourse.mybir as mybir
from concourse import ap_utils, bass_utils
from concourse.bass import exact_div
from concourse.library_config import mlp
from concourse.library_overlay import lower_extended_insts

F32 = mybir.dt.float32
F16 = mybir.dt.float16
I16 = mybir.dt.int16
AX = mybir.AxisListType
ALU = mybir.AluOpType
ACT = mybir.ActivationFunctionType

GAMMA = 10.0
N_CORES = 8
P = 128            # partitions
PINS_PER_ROW = 16  # fp16 pin pairs per 256B-strided t1 row
T1_BASE = 32768    # in_ap base row; idx = row - T1_BASE (signed int16)
HIGH_PIN = T1_BASE * PINS_PER_ROW  # pins >= this have idx >= 0
MAX_COLS = 120     # slot columns per compute chunk
SUB_COLS = 8       # columns per gather instruction (1024-desc ring cap)


def dma_gather_raw(gp, out_ap, in_ap, idxs_ap, num_idxs, elem_size,
                   elem_step, queue_num=0):
    """bass.BassGpSimd.dma_gather without the elem_size%256 assert (the
    non-transpose HBM ucode only requires the row STRIDE to be a multiple
    of 256B) and with num_idxs_reg pinned to num_idxs so the decode-side
    ring accounting covers inner negative indices."""
    assert idxs_ap.dtype == mybir.dt.int16
    assert in_ap.dtype == out_ap.dtype
    assert ap_utils.ap_is_contiguous(in_ap.ap[1:])
    assert ap_utils.ap_is_contiguous(out_ap.ap[1:])
    assert ap_utils.ap_is_contiguous(idxs_ap.ap[1:])
    assert in_ap.ap[-1][1] == elem_size
    assert out_ap.ap[-1][1] == elem_size
    assert out_ap.ap[0][1] * out_ap.ap[1][1] == ((num_idxs + 127) // 128) * 128
    assert in_ap.ap[0][0] == elem_step
    assert num_idxs <= 1024  # SWDGE descriptor-ring cap per instruction
    stride_bytes_256 = exact_div(elem_step * mybir.dt.size(in_ap.dtype), 256)
    assert 0 < stride_bytes_256 < 256

    cache = getattr(gp, "_gather_reg_cache", None)
    if cache is None:
        cache = gp._gather_reg_cache = {}
    if num_idxs not in cache:
        cache[num_idxs] = gp.lower_val_access(gp.to_reg(num_idxs))
    _in_ap = gp.lower_ap_dma(in_ap, for_custom_bir_dma=True)
    return gp.add_instruction(
        mybir.InstDMAGatherAnt(
            name=gp.bass.get_next_instruction_name(),
            ins=[
                *_in_ap,
                gp.lower_ap(idxs_ap),
                cache[num_idxs],
            ],
            outs=[gp.lower_ap(out_ap)],
            transpose=False,
            num_idxs=num_idxs,
            elem_size=elem_size,
            stride_bytes_256=stride_bytes_256,
            gen_mode=0,
            single_packet=True,
            queue_num=queue_num,
            sbuf_tokens_per_rank=0,
            sbuf_free_dim_per_rank=0,
            sbuf_free_dim_pad_per_rank=0,
            sbuf_byte_offset=0,
        )
    )


DEBUG_MODE = "full"  # full | build | gather | seg1


def build_program(vpad, n_t1_rows, chunk_plan, tot_slot, tot_g):
    """chunk_plan: list of (k, g, slot_off, g_off); cs = g*k slot columns."""
    mode = DEBUG_MODE
    nc = bass.Bass("TRN2", target_bir_lowering=False, debug=False,
                   num_swdge_queues=4)
    nt = vpad // P

    t1 = nc.dram_tensor("t1", [n_t1_rows, P], F16, kind="ExternalInput")
    posxy = nc.dram_tensor("posxy", [vpad, 16], F32, kind="ExternalInput")
    oh = nc.dram_tensor("oh", [vpad, 32], F32, kind="ExternalInput")
    idx1d = nc.dram_tensor("idx1", [P, tot_slot * 8], I16, kind="ExternalInput")
    idx2d = nc.dram_tensor("idx2", [P, tot_slot * 8], I16, kind="ExternalInput")
    maskd = nc.dram_tensor("maskt", [P, tot_slot * 16], F16, kind="ExternalInput")
    wd = nc.dram_tensor("w_all", [P, tot_g], F32, kind="ExternalInput")
    t2 = nc.dram_tensor("t2", [vpad, P], F16, kind="Internal")

    nch = len(chunk_plan)
    CS = MAX_COLS
    dump = None
    if mode.startswith("repeat:"):
        parts_ = mode.split(":")
        reps = int(parts_[1])
        chunk_plan = tuple(chunk_plan) * reps
        nch = len(chunk_plan)
        mode = parts_[2] if len(parts_) > 2 else "full"
    if mode.startswith("trunc:"):
        n = int(mode.split(":")[1])
        chunk_plan = chunk_plan[:n]
        nch = n
        mode = "full"
    dump_ci = 0
    if mode.startswith("dump"):
        if ":" in mode:
            dump_ci = int(mode.split(":")[1])
        chunk_plan = chunk_plan[:dump_ci + 1]
        nch = dump_ci + 1
    out = nc.dram_tensor("acc", [P, tot_g * 8], F32, kind="ExternalOutput")
    if mode.startswith("dump"):
        dump = {
            "d_rec1": nc.dram_tensor("d_rec1", [P, CS * 32], F16,
                                     kind="ExternalOutput"),
            "d_rec2": nc.dram_tensor("d_rec2", [P, CS * 32], F16,
                                     kind="ExternalOutput"),
            "d_t1x": nc.dram_tensor("d_t1x", [P, CS * 2], F32,
                                    kind="ExternalOutput"),
            "d_pv": nc.dram_tensor("d_pv", [P, CS * 16], F16,
                                   kind="ExternalOutput"),
            "d_dmx": nc.dram_tensor("d_dmx", [P, CS * 16], F16,
                                    kind="ExternalOutput"),
            "d_dmn": nc.dram_tensor("d_dmn", [P, CS * 16], F16,
                                    kind="ExternalOutput"),
            "d_sx": nc.dram_tensor("d_sx", [P, CS * 8], F32,
                                   kind="ExternalOutput"),
            "d_sn": nc.dram_tensor("d_sn", [P, CS * 8], F32,
                                   kind="ExternalOutput"),
            "d_t2": nc.dram_tensor("d_t2", [vpad, 32], F16,
                                   kind="ExternalOutput"),
            "d_wch": nc.dram_tensor("d_wch", [P, CS * 8], F32,
                                    kind="ExternalOutput"),
            "d_wnb": nc.dram_tensor("d_wnb", [P, CS * 4], F32,
                                    kind="ExternalOutput"),
            "d_part": nc.dram_tensor("d_part", [P, 8], F32,
                                     kind="ExternalOutput"),
            "d_mx": nc.dram_tensor("d_mx", [P, CS * 8], F16,
                                   kind="ExternalOutput"),
        }

    import contextlib
    with contextlib.ExitStack() as st:
        sb = lambda name, shape, dtype: st.enter_context(
            nc.sbuf_tensor(name, shape, dtype))
        sem = lambda name: st.enter_context(nc.semaphore(name))

        w_sb = sb("w_sb", [P, tot_g], F32)
        parts = sb("parts_sb", [P, tot_g * 8], F32)
        idx1_sb = [sb(f"idx1_{i}", [P, CS * 8], I16) for i in range(2)]
        idx2_sb = [sb(f"idx2_{i}", [P, CS * 8], I16) for i in range(2)]
        mask_sb = [sb(f"mask_{i}", [P, CS * 16], F16) for i in range(2)]
        rec1 = [sb(f"rec1_{i}", [P, CS * 32], F16) for i in range(2)]
        rec2 = [sb(f"rec2_{i}", [P, CS * 32], F16) for i in range(2)]
        tmpe = [sb(f"tmpe_{i}", [P, CS * 32], F16) for i in range(2)]
        t1x = [sb(f"t1x_{i}", [P, CS * 2], F32) for i in range(2)]
        pv = [sb(f"pv_{i}", [P, CS * 16], F16) for i in range(2)]
        ta = [sb(f"ta_{i}", [P, CS * 8], F16) for i in range(2)]
        tb = [sb(f"tb_{i}", [P, CS * 8], F16) for i in range(2)]
        dmx = [sb(f"dmx_{i}", [P, CS * 16], F16) for i in range(2)]
        dmn = [sb(f"dmn_{i}", [P, CS * 16], F16) for i in range(2)]
        Mx = [sb(f"Mx_{i}", [P, CS * 8], F16) for i in range(2)]
        mn = [sb(f"mn_{i}", [P, CS * 8], F16) for i in range(2)]
        Sx = [sb(f"Sx_{i}", [P, CS * 8], F32) for i in range(2)]
        Sn = [sb(f"Sn_{i}", [P, CS * 8], F32) for i in range(2)]
        wch = [sb(f"wch_{i}", [P, CS * 8], F32) for i in range(2)]
        wnb = [sb(f"wnb_{i}", [P, CS * 4], F32) for i in range(2)]

        s_io = [sem(f"s_io{i}") for i in range(2)]
        s_g1 = [sem(f"s_g1{i}") for i in range(2)]
        s_g2 = [sem(f"s_g2{i}") for i in range(2)]
        s_cons = [sem(f"s_cons{i}") for i in range(2)]
        s_build = sem("s_build")
        s_d1 = sem("s_d1")
        s_act1 = sem("s_act1")
        s_d2 = sem("s_d2")
        s_act2 = sem("s_act2")
        s_fin = sem("s_fin")
        all_sems = (s_io + s_g1 + s_g2 + s_cons
                    + [s_build, s_d1, s_act1, s_d2, s_act2, s_fin])

        nc.gpsimd.load_library(mlp)

        # ---- T2 build ----
        with (
            nc.sbuf_tensor("pos_sb", [P, nt * 16], F32) as pos_sb,
            nc.sbuf_tensor("oh_sb", [P, nt * 32], F32) as oh_sb,
            nc.sbuf_tensor("ohs_sb", [P, nt * 32], F32) as ohs_sb,
            nc.sbuf_tensor("t2img", [P, nt * 32], F16) as t2img,
        ):
            nc.sync.dma_start(
                pos_sb[:].rearrange("p (t f) -> p t f", t=nt),
                posxy.ap().rearrange("(t p) f -> p t f", p=P),
            ).then_inc(s_build, 16)
            nc.sync.dma_start(
                oh_sb[:].rearrange("p (t f) -> p t f", t=nt),
                oh.ap().rearrange("(t p) f -> p t f", p=P),
            ).then_inc(s_build, 16)
            nc.sync.dma_start(w_sb[:], wd.ap()).then_inc(s_build, 16)

            t2r = t2img[:].rearrange("p (t c) -> p t c", t=nt)
            ohr = ohs_sb[:].rearrange("p (t b f) -> p t b f", t=nt, f=4)
            nc.scalar.wait_ge(s_build, 48)
            nc.scalar.activation(out=ohs_sb[:], in_=oh_sb[:], func=ACT.Copy,
                                 scale=GAMMA)
            nc.scalar.activation(out=t2r[:, :, 0:16],
                                 in_=pos_sb[:].rearrange("p (t f) -> p t f", t=nt),
                                 func=ACT.Copy, scale=GAMMA).then_inc(s_d1, 1)
            nc.vector.wait_ge(s_d1, 1)
            nc.vector.tensor_tensor(out=t2r[:, :, 16:24], in0=ohr[:, :, :, 0],
                                    in1=ohr[:, :, :, 2], op=ALU.subtract)
            nc.vector.tensor_tensor(out=t2r[:, :, 24:32], in0=ohr[:, :, :, 1],
                                    in1=ohr[:, :, :, 3], op=ALU.subtract)
            nc.vector.memset(parts[:], 0.0).then_inc(s_d1, 1)
            nc.sync.wait_ge(s_d1, 2)
            nc.sync.dma_start(
                t2.ap().rearrange("(t p) c -> p t c", p=P)[:, :, 0:32], t2r
            ).then_inc(s_build, 16)

        t1_in = t1.ap()[T1_BASE:, 0:32]
        t2_in = t2.ap()[:, 0:32]

        # ---- chunk pipeline ----
        # Software-pipelined: chunk c's seg2/seg3 (and ACT ln) are emitted
        # after chunk c+1's seg1/exp so the DVE never stalls on the ACT
        # round-trips. All semaphore targets are by chunk index.
        ng = [0, 0]   # cumulative gather count per parity slot
        plan = chunk_plan if mode != "build" else []

        def emit_head(c, k, g, slot_off, g_off):
            cs = g * k
            b = c % 2
            if c >= 2:
                nc.sync.wait_ge(s_cons[b], c // 2)
            nc.sync.dma_start(
                idx1_sb[b][:, 0:cs * 8],
                idx1d.ap()[:, slot_off * 8:(slot_off + cs) * 8],
            ).then_inc(s_io[b], 16)
            nc.sync.dma_start(
                idx2_sb[b][:, 0:cs * 8],
                idx2d.ap()[:, slot_off * 8:(slot_off + cs) * 8],
            ).then_inc(s_io[b], 16)
            nc.sync.dma_start(
                mask_sb[b][:, 0:cs * 16],
                maskd.ap()[:, slot_off * 16:(slot_off + cs) * 16],
            ).then_inc(s_io[b], 16)

            nc.gpsimd.wait_ge(s_io[b], 48 * (c // 2 + 1))
            if c == 0:
                nc.gpsimd.wait_ge(s_build, 64)
            if c >= 2:
                nc.gpsimd.wait_ge(s_cons[b], c // 2)
            r1 = rec1[b][:].rearrange("p (n e) -> p n e", e=32)
            r2 = rec2[b][:].rearrange("p (n e) -> p n e", e=32)
            for s0 in range(0, cs, SUB_COLS):
                s1 = min(s0 + SUB_COLS, cs)
                nidx = (s1 - s0) * P
                dma_gather_raw(nc.gpsimd, r1[:, s0:s1, :], t1_in,
                               idx1_sb[b][:, s0 * 8:s1 * 8], nidx, 32, P,
                               queue_num=(2 * c) % 4).then_inc(s_g1[b], 16)
                dma_gather_raw(nc.gpsimd, r2[:, s0:s1, :], t2_in,
                               idx2_sb[b][:, s0 * 8:s1 * 8], nidx, 32, P,
                               queue_num=(2 * c + 1) % 4).then_inc(s_g2[b], 16)
                ng[b] += 1

            if mode == "gather":
                nc.vector.wait_ge(s_g1[b], 16 * ng[b])
                nc.vector.wait_ge(s_g2[b], 16 * ng[b])
                nc.vector.sem_inc(s_cons[b], 1)
                nc.vector.sem_inc(s_fin, 1)
                return

            # DVE seg1: extract + rotate + max/min + shifts
            nc.vector.wait_ge(s_g1[b], 16 * ng[b])
            nc.vector.wait_ge(s_io[b], 48 * (c // 2 + 1))
            r1v = rec1[b][:].rearrange("p (n j two) -> p n two j", two=2,
                                       j=PINS_PER_ROW)[:, 0:cs, :, :]
            mkb = (mask_sb[b][:].rearrange("p (n j) -> p n j", j=16)
                   [:, 0:cs, :].unsqueeze(2).to_broadcast([P, cs, 2, 16]))
            tev = tmpe[b][:].rearrange("p (n t j) -> p n t j", t=2, j=16)[:, 0:cs]
            nc.vector.tensor_tensor(out=tev, in0=r1v, in1=mkb, op=ALU.mult)
            nc.vector.tensor_reduce(
                out=t1x[b][:, 0:cs * 2],
                in_=tmpe[b][:].rearrange("p (s j) -> p s j", j=16)[:, 0:cs * 2],
                axis=AX.X, op=ALU.add)

            nc.vector.wait_ge(s_g2[b], 16 * ng[b])
            r5 = r2[:, 0:cs, :].rearrange("p (g j) c -> p g j c", g=g)
            Xv = r5[:, :, :, 0:8]
            Yv = r5[:, :, :, 8:16]
            Cv = r5[:, :, :, 16:24]
            Sv = r5[:, :, :, 24:32]
            t1r = t1x[b][:, 0:cs * 2].rearrange("p (g j two) -> p g j two",
                                                g=g, two=2)
            oxv = t1r[:, :, :, 0:1].to_broadcast([P, g, k, 8])
            oyv = t1r[:, :, :, 1:2].to_broadcast([P, g, k, 8])
            pvr = pv[b][:, 0:cs * 16].rearrange("p (g b c j) -> p g b c j",
                                                g=g, b=8, c=2)
            pxo = pvr[:, :, :, 0, :].transpose([0, 1, 3, 2])
            pyo = pvr[:, :, :, 1, :].transpose([0, 1, 3, 2])
            tar = ta[b][:, 0:cs * 8].rearrange("p (g j b) -> p g j b", g=g, b=8)
            tbr = tb[b][:, 0:cs * 8].rearrange("p (g j b) -> p g j b", g=g, b=8)

            nc.vector.tensor_tensor(out=tar, in0=Cv, in1=oxv, op=ALU.mult)
            nc.vector.tensor_tensor(out=tbr, in0=Sv, in1=oyv, op=ALU.mult)
            nc.vector.tensor_tensor(out=tar, in0=tar, in1=Xv, op=ALU.add)
            nc.vector.tensor_tensor(out=pxo, in0=tar, in1=tbr, op=ALU.subtract)
            nc.vector.tensor_tensor(out=tar, in0=Sv, in1=oxv, op=ALU.mult)
            nc.vector.tensor_tensor(out=tbr, in0=Cv, in1=oyv, op=ALU.mult)
            nc.vector.tensor_tensor(out=tar, in0=tar, in1=Yv, op=ALU.add)
            nc.vector.tensor_tensor(out=pyo, in0=tar, in1=tbr,
                                    op=ALU.add).then_inc(s_cons[b], 1)

            nn = g * 16
            pvs = pv[b][:, 0:cs * 16].rearrange("p (s j) -> p s j", j=k)
            nc.vector.tensor_reduce(out=Mx[b][:, 0:nn], in_=pvs, axis=AX.X,
                                    op=ALU.max)
            nc.vector.tensor_reduce(out=mn[b][:, 0:nn], in_=pvs, axis=AX.X,
                                    op=ALU.min)
            Mb = Mx[b][:, 0:nn].unsqueeze(2).to_broadcast([P, nn, k])
            mb = mn[b][:, 0:nn].unsqueeze(2).to_broadcast([P, nn, k])
            dxr = dmx[b][:, 0:cs * 16].rearrange("p (s j) -> p s j", j=k)
            dnr = dmn[b][:, 0:cs * 16].rearrange("p (s j) -> p s j", j=k)
            nc.vector.tensor_tensor(out=dxr, in0=pvs, in1=Mb, op=ALU.subtract)
            nc.vector.tensor_tensor(out=dnr, in0=pvs, in1=mb,
                                    op=ALU.subtract).then_inc(s_d1, 1)

            if mode == "seg1":
                nc.vector.sem_inc(s_fin, 1)
                return
            # ACT: exps (in place)
            nc.scalar.wait_ge(s_d1, 3 + c)
            nc.scalar.activation(out=dmx[b][:, 0:cs * 16],
                                 in_=dmx[b][:, 0:cs * 16], func=ACT.Exp)
            nc.scalar.activation(out=dmn[b][:, 0:cs * 16],
                                 in_=dmn[b][:, 0:cs * 16], func=ACT.Exp,
                                 scale=-1.0).then_inc(s_act1, 1)

        def emit_tail(c, k, g, slot_off, g_off):
            cs = g * k
            b = c % 2
            nn = g * 16
            dxr = dmx[b][:, 0:cs * 16].rearrange("p (s j) -> p s j", j=k)
            dnr = dmn[b][:, 0:cs * 16].rearrange("p (s j) -> p s j", j=k)
            # DVE seg2: sums + extent
            nc.vector.wait_ge(s_act1, c + 1)
            nc.vector.tensor_reduce(out=Sx[b][:, 0:nn], in_=dxr, axis=AX.X,
                                    op=ALU.add)
            nc.vector.tensor_reduce(out=Sn[b][:, 0:nn], in_=dnr, axis=AX.X,
                                    op=ALU.add)
            nc.vector.tensor_tensor(out=wch[b][:, 0:nn], in0=Mx[b][:, 0:nn],
                                    in1=mn[b][:, 0:nn],
                                    op=ALU.subtract).then_inc(s_d2, 1)
            # ACT: logs (in place)
            nc.scalar.wait_ge(s_d2, c + 1)
            nc.scalar.activation(out=Sx[b][:, 0:nn], in_=Sx[b][:, 0:nn],
                                 func=ACT.Ln)
            nc.scalar.activation(out=Sn[b][:, 0:nn], in_=Sn[b][:, 0:nn],
                                 func=ACT.Ln).then_inc(s_act2, 1)
            # DVE seg3: combine + weight into parts slice
            nc.vector.wait_ge(s_act2, c + 1)
            nc.vector.tensor_tensor(out=wch[b][:, 0:nn], in0=wch[b][:, 0:nn],
                                    in1=Sx[b][:, 0:nn], op=ALU.add)
            nc.vector.tensor_tensor(out=wch[b][:, 0:nn], in0=wch[b][:, 0:nn],
                                    in1=Sn[b][:, 0:nn], op=ALU.add)
            pslice = parts[:, g_off * 8:(g_off + g) * 8]
            nc.vector.tensor_reduce(
                out=pslice,
                in_=wch[b][:, 0:nn].rearrange("p (s c) -> p s c", c=2),
                axis=AX.X, op=ALU.add)
            wbr = (w_sb[:, g_off:g_off + g].unsqueeze(2)
                   .to_broadcast([P, g, 8]))
            psv = pslice.rearrange("p (g b) -> p g b", g=g)
            nc.vector.tensor_tensor(out=psv, in0=psv, in1=wbr,
                                    op=ALU.mult).then_inc(s_fin, 1)

        prev = None
        for c, ck in enumerate(plan):
            emit_head(c, *ck)
            if mode in ("gather", "seg1"):
                continue
            if prev is not None:
                emit_tail(prev[0], *prev[1])
            prev = (c, ck)
        if prev is not None and mode not in ("gather", "seg1"):
            emit_tail(prev[0], *prev[1])

        # ---- writeback + cleanup ----
        if mode != "build":
            nc.sync.wait_ge(s_fin, nch)
        nfin = 80
        if dump is not None:
            dp = dump_ci % 2
            nc.sync.dma_start(dump["d_rec1"].ap(), rec1[dp][:]).then_inc(s_build, 16)
            nc.sync.dma_start(dump["d_rec2"].ap(), rec2[dp][:]).then_inc(s_build, 16)
            nc.sync.dma_start(dump["d_t1x"].ap(), t1x[dp][:]).then_inc(s_build, 16)
            nc.sync.dma_start(dump["d_pv"].ap(), pv[dp][:]).then_inc(s_build, 16)
            nc.sync.dma_start(dump["d_dmx"].ap(), dmx[dp][:]).then_inc(s_build, 16)
            nc.sync.dma_start(dump["d_dmn"].ap(), dmn[dp][:]).then_inc(s_build, 16)
            nc.sync.dma_start(dump["d_sx"].ap(), Sx[dp][:]).then_inc(s_build, 16)
            nc.sync.dma_start(dump["d_sn"].ap(), Sn[dp][:]).then_inc(s_build, 16)
            nc.sync.dma_start(dump["d_t2"].ap(),
                              t2.ap()[:, 0:32]).then_inc(s_build, 16)
            nc.sync.dma_start(dump["d_wch"].ap(), wch[dp][:]).then_inc(s_build, 16)
            nc.sync.dma_start(dump["d_wnb"].ap(), wnb[dp][:]).then_inc(s_build, 16)
            nc.sync.dma_start(dump["d_part"].ap(),
                              parts[:, 0:8]).then_inc(s_build, 16)
            nc.sync.dma_start(dump["d_mx"].ap(), Mx[dp][:]).then_inc(s_build, 16)
            nfin = 288
        nc.sync.dma_start(out.ap(), parts[:]).then_inc(s_build, 16)
        nc.sync.wait_ge(s_build, nfin)
        nc.all_engine_barrier()
        for s in all_sems:
            nc.gpsimd.sem_clear(s)
        nc.all_engine_barrier()

    lower_extended_insts(nc)
    return nc


def prep_host(positions, pin_offsets, rotation_onehot, net_weights,
              net_to_pin, pin_to_macro):
    """Host-side sharding + metadata layout. Returns (meta, in_maps)."""
    B, V, _ = positions.shape
    Pn = pin_offsets.shape[0]
    N, M = net_to_pin.shape

    vpad = ((V + 1 + P - 1) // P) * P
    pad_mac = V
    n_t1_rows = (Pn + PINS_PER_ROW - 1) // PINS_PER_ROW

    n2p = net_to_pin.astype(np.int32)
    p2m = pin_to_macro.astype(np.int32)

    t1 = np.zeros((n_t1_rows, P), np.float16)
    t1[:, 0:32] = pin_offsets.astype(np.float16).reshape(n_t1_rows, 32)
    posxy = np.zeros((vpad, 16), np.float32)
    posxy[:V, 0:8] = positions[:, :, 0].T
    posxy[:V, 8:16] = positions[:, :, 1].T
    oh = np.zeros((vpad, 32), np.float32)
    oh[:V] = rotation_onehot.transpose(1, 0, 2).reshape(V, 4 * B)

    lengths = (n2p >= 0).sum(axis=1)

    per = (N + N_CORES - 1) // N_CORES
    shards = [(c * per, min((c + 1) * per, N)) for c in range(N_CORES)]

    counts = np.zeros((N_CORES, M + 1), np.int64)
    for c, (a, b) in enumerate(shards):
        counts[c] = np.bincount(lengths[a:b], minlength=M + 1)
    gk = {k: int(-(-counts[:, k].max() // P))
          for k in range(1, M + 1) if counts[:, k].max() > 0}

    chunk_plan = []
    slot_off = 0
    g_off = 0
    bucket_offs = {}
    for k in sorted(gk):
        g_total = gk[k]
        gmax = max(1, MAX_COLS // k)
        bucket_offs[k] = (slot_off, g_off)
        nsplit = -(-g_total // gmax)
        g_per = -(-g_total // nsplit)  # balanced split, no tiny tails
        g_done = 0
        while g_done < g_total:
            g = min(g_per, g_total - g_done)
            chunk_plan.append((k, g, slot_off, g_off))
            slot_off += g * k
            g_off += g
            g_done += g
    tot_slot = slot_off
    tot_g = g_off

    # global columns where a pin sub-gather ends: last column of each
    # SUB_COLS block within each chunk (plus the chunk tail)
    boundary_cols = set()
    for (k, g, so, go) in chunk_plan:
        cs = g * k
        for s0 in range(0, cs, SUB_COLS):
            boundary_cols.add(so + min(s0 + SUB_COLS, cs) - 1)

    def wrap16(vals):
        L = vals.shape[0] // 16
        w = vals.reshape(L, 16).T.astype(np.int16)
        return np.tile(w, (8, 1))

    in_maps = []
    rng = np.random.default_rng(12345)
    for c, (a, b) in enumerate(shards):
        pin_t = np.full((P, tot_slot), HIGH_PIN, np.int32)
        mac_t = np.full((P, tot_slot), pad_mac, np.int32)
        w_all = np.zeros((P, tot_g), np.float32)
        ln = lengths[a:b]
        for k in sorted(gk):
            so, go = bucket_offs[k]
            sel = np.nonzero(ln == k)[0]
            nk = len(sel)
            gkk = gk[k]
            ids = np.full((gkk * P, k), HIGH_PIN, np.int32)
            wp_ = np.zeros((gkk * P,), np.float32)
            if nk:
                ids[:nk] = n2p[a:b][sel][:, :k]
                wp_[:nk] = net_weights[a:b][sel].astype(np.float32) / GAMMA

            # fix sub-gather boundary positions: net at (g_loc, p=127) must
            # have pins >= HIGH_PIN at every boundary j of its group
            for g_loc in range(gkk):
                js = [(col - so) % k for col in range(so + g_loc * k,
                                                      so + (g_loc + 1) * k)
                      if col in boundary_cols]
                if not js:
                    continue
                row127 = g_loc * P + 127
                lo = g_loc * P
                hi = min(g_loc * P + P, gkk * P)
                cand_rows = None
                cur = ids[row127] if row127 < gkk * P else None
                if cur is not None and (cur >= HIGH_PIN).sum() >= len(js):
                    cand_rows = row127
                else:
                    high_counts = (ids[lo:hi] >= HIGH_PIN).sum(axis=1)
                    ok_rows = np.nonzero(high_counts >= len(js))[0]
                    assert len(ok_rows) > 0, (
                        f"no boundary-safe net in bucket k={k} group {g_loc}")
                    cand_rows = lo + ok_rows[0]
                if cand_rows != row127:
                    ids[[row127, cand_rows]] = ids[[cand_rows, row127]]
                    wp_[[row127, cand_rows]] = wp_[[cand_rows, row127]]
                # reorder pins within the row127 net: place high pins at js
                row = ids[row127].copy()
                high = np.nonzero(row >= HIGH_PIN)[0]
                rest = [i for i in range(k) if i not in set(high[:len(js)])]
                newrow = np.empty(k, np.int32)
                used = set()
                for j, hsrc in zip(js, high):
                    newrow[j] = row[hsrc]
                    used.add(hsrc)
                fill = [row[i] for i in range(k) if i not in used]
                fi = 0
                for j in range(k):
                    if j not in set(js):
                        newrow[j] = fill[fi]
                        fi += 1
                ids[row127] = newrow

            pin_t[:, so:so + gkk * k] = (
                ids.reshape(gkk, P, k).transpose(1, 0, 2).reshape(P, gkk * k))
            w_all[:, go:go + gkk] = wp_.reshape(gkk, P).T

        valid_t = pin_t < Pn
        mac_t = np.where(valid_t, p2m[np.where(valid_t, pin_t, 0)], pad_mac)

        idx1 = np.zeros((P, tot_slot * 8), np.int16)
        idx2 = np.zeros((P, tot_slot * 8), np.int16)
        for (k, g, so, go) in chunk_plan:
            cs = g * k
            blk_pin = pin_t[:, so:so + cs].T.reshape(-1)
            blk_mac = mac_t[:, so:so + cs].T.reshape(-1)
            idx1[:, so * 8:(so + cs) * 8] = wrap16(
                blk_pin // PINS_PER_ROW - T1_BASE)
            idx2[:, so * 8:(so + cs) * 8] = wrap16(blk_mac)

        maskt = (np.arange(16)[None, None, :]
                 == (pin_t % PINS_PER_ROW)[:, :, None]).astype(np.float16)
        in_maps.append({
            "t1": t1, "posxy": posxy, "oh": oh,
            "idx1": idx1, "idx2": idx2,
            "maskt": maskt.reshape(P, tot_slot * 16),
            "w_all": w_all,
        })

    meta = (vpad, n_t1_rows, tuple(chunk_plan), tot_slot, tot_g)
    return meta, in_maps


_prog_cache = {}


def kernel(**inputs):
    meta, in_maps = prep_host(
        np.asarray(inputs["positions"]),
        np.asarray(inputs["pin_offsets"]),
        np.asarray(inputs["rotation_onehot"]),
        np.asarray(inputs["net_weights"]),
        np.asarray(inputs["net_to_pin"]),
        np.asarray(inputs["pin_to_macro"]),
    )
    if meta not in _prog_cache:
        _prog_cache[meta] = build_program(*meta)
    nc = _prog_cache[meta]
    res = bass_utils.run_bass_kernel_spmd(nc, in_maps, core_ids=list(range(N_CORES)))
    total = np.zeros(8, np.float64)
    for r in res.results:
        a = r["acc"].astype(np.float64)
        total += a.reshape(a.shape[0], -1, 8).sum(axis=(0, 1))
    return total.astype(np.float32)


# revision 29
# speedup vs baseline: 2.7123x; 1.1365x over previous
"""DifferentiableHPWL on 8 trn2 NeuronCores — dma_gather version.

Strategy (sharded by nets per the hint):
  - Host (metadata/layout only): bucket nets by pin count, shard nets
    across 8 cores, compose per-slot int16 gather-index tables for
    InstDMAGatherAnt, one-hot pin-extract masks, and pack pin_offsets
    into a 256B-strided fp16 table (16 pins/row).
  - Device per core:
      * build macro record table T2[m] = gamma*(x[8],y[8],c[8],s[8]) fp16
        (64B payload / 256B stride) from positions + rotation_onehot
      * per compute chunk of <=120 slot columns: 8-column (1024-index,
        the SWDGE ring cap) dma_gather sub-instructions for macro
        records (int16 macro ids) and pin-offset blocks (idx =
        pin//16 - 32768 against a base-shifted ap so signed int16
        covers all 50000 rows), then DVE: mask-extract the pin pair,
        rotate, per-net logsumexp max/min (gamma prefolded), weighted
        accumulate into acc[128, 8].
  - Host: sum partials over partitions and cores.

The SWDGE ucode trims TRAILING negative indices per instruction, so the
host guarantees each pin sub-gather's final position (partition 127 of
its last column) holds a pin >= 16*32768: pad nets use pin 524288 and
real nets are swapped/pin-reordered per group as needed (P(impossible)
is cryptographically small; asserted).
"""

import numpy as np

import concourse.bass as bass
import concourse.mybir as mybir
from concourse import ap_utils, bass_utils
from concourse.bass import exact_div
from concourse.library_config import mlp
from concourse.library_overlay import lower_extended_insts

F32 = mybir.dt.float32
F16 = mybir.dt.float16
I16 = mybir.dt.int16
AX = mybir.AxisListType
ALU = mybir.AluOpType
ACT = mybir.ActivationFunctionType

GAMMA = 10.0
N_CORES = 8
P = 128            # partitions
PINS_PER_ROW = 16  # fp16 pin pairs per 256B-strided t1 row
T1_BASE = 32768    # in_ap base row; idx = row - T1_BASE (signed int16)
HIGH_PIN = T1_BASE * PINS_PER_ROW  # pins >= this have idx >= 0
MAX_COLS = 120     # slot columns per compute chunk
SUB_COLS = 8       # columns per gather instruction (1024-desc ring cap)
SINGLE_PACKET = True


def dma_gather_raw(gp, out_ap, in_ap, idxs_ap, num_idxs, elem_size,
                   elem_step, queue_num=0):
    """bass.BassGpSimd.dma_gather without the elem_size%256 assert (the
    non-transpose HBM ucode only requires the row STRIDE to be a multiple
    of 256B) and with num_idxs_reg pinned to num_idxs so the decode-side
    ring accounting covers inner negative indices."""
    assert idxs_ap.dtype == mybir.dt.int16
    assert in_ap.dtype == out_ap.dtype
    assert ap_utils.ap_is_contiguous(in_ap.ap[1:])
    assert ap_utils.ap_is_contiguous(out_ap.ap[1:])
    assert ap_utils.ap_is_contiguous(idxs_ap.ap[1:])
    assert in_ap.ap[-1][1] == elem_size
    assert out_ap.ap[-1][1] == elem_size
    assert out_ap.ap[0][1] * out_ap.ap[1][1] == ((num_idxs + 127) // 128) * 128
    assert in_ap.ap[0][0] == elem_step
    assert num_idxs <= 1024  # SWDGE descriptor-ring cap per instruction
    stride_bytes_256 = exact_div(elem_step * mybir.dt.size(in_ap.dtype), 256)
    assert 0 < stride_bytes_256 < 256

    cache = getattr(gp, "_gather_reg_cache", None)
    if cache is None:
        cache = gp._gather_reg_cache = {}
    if num_idxs not in cache:
        cache[num_idxs] = gp.lower_val_access(gp.to_reg(num_idxs))
    _in_ap = gp.lower_ap_dma(in_ap, for_custom_bir_dma=True)
    return gp.add_instruction(
        mybir.InstDMAGatherAnt(
            name=gp.bass.get_next_instruction_name(),
            ins=[
                *_in_ap,
                gp.lower_ap(idxs_ap),
                cache[num_idxs],
            ],
            outs=[gp.lower_ap(out_ap)],
            transpose=False,
            num_idxs=num_idxs,
            elem_size=elem_size,
            stride_bytes_256=stride_bytes_256,
            gen_mode=0,
            single_packet=SINGLE_PACKET,
            queue_num=queue_num,
            sbuf_tokens_per_rank=0,
            sbuf_free_dim_per_rank=0,
            sbuf_free_dim_pad_per_rank=0,
            sbuf_byte_offset=0,
        )
    )


DEBUG_MODE = "full"  # full | build | gather | seg1


def build_program(vpad, n_t1_rows, chunk_plan, tot_slot, tot_g):
    """chunk_plan: list of (k, g, slot_off, g_off); cs = g*k slot columns."""
    mode = DEBUG_MODE
    nc = bass.Bass("TRN2", target_bir_lowering=False, debug=False,
                   num_swdge_queues=4)
    nt = vpad // P

    t1 = nc.dram_tensor("t1", [n_t1_rows, P], F16, kind="ExternalInput")
    posxy = nc.dram_tensor("posxy", [vpad, 16], F32, kind="ExternalInput")
    oh = nc.dram_tensor("oh", [vpad, 32], F32, kind="ExternalInput")
    idx1d = nc.dram_tensor("idx1", [P, tot_slot * 8], I16, kind="ExternalInput")
    idx2d = nc.dram_tensor("idx2", [P, tot_slot * 8], I16, kind="ExternalInput")
    maskd = nc.dram_tensor("maskt", [P, tot_slot * 16], F16, kind="ExternalInput")
    wd = nc.dram_tensor("w_all", [P, tot_g], F32, kind="ExternalInput")
    t2 = nc.dram_tensor("t2", [vpad, P], F16, kind="Internal")

    nch = len(chunk_plan)
    CS = MAX_COLS
    dump = None
    if mode.startswith("repeat:"):
        parts_ = mode.split(":")
        reps = int(parts_[1])
        chunk_plan = tuple(chunk_plan) * reps
        nch = len(chunk_plan)
        mode = parts_[2] if len(parts_) > 2 else "full"
    if mode.startswith("trunc:"):
        n = int(mode.split(":")[1])
        chunk_plan = chunk_plan[:n]
        nch = n
        mode = "full"
    dump_ci = 0
    if mode.startswith("dump"):
        if ":" in mode:
            dump_ci = int(mode.split(":")[1])
        chunk_plan = chunk_plan[:dump_ci + 1]
        nch = dump_ci + 1
    out = nc.dram_tensor("acc", [P, tot_g * 8], F32, kind="ExternalOutput")
    if mode.startswith("dump"):
        dump = {
            "d_rec1": nc.dram_tensor("d_rec1", [P, CS * 32], F16,
                                     kind="ExternalOutput"),
            "d_rec2": nc.dram_tensor("d_rec2", [P, CS * 32], F16,
                                     kind="ExternalOutput"),
            "d_t1x": nc.dram_tensor("d_t1x", [P, CS * 2], F32,
                                    kind="ExternalOutput"),
            "d_pv": nc.dram_tensor("d_pv", [P, CS * 16], F16,
                                   kind="ExternalOutput"),
            "d_dmx": nc.dram_tensor("d_dmx", [P, CS * 16], F16,
                                    kind="ExternalOutput"),
            "d_dmn": nc.dram_tensor("d_dmn", [P, CS * 16], F16,
                                    kind="ExternalOutput"),
            "d_sx": nc.dram_tensor("d_sx", [P, CS * 8], F32,
                                   kind="ExternalOutput"),
            "d_sn": nc.dram_tensor("d_sn", [P, CS * 8], F32,
                                   kind="ExternalOutput"),
            "d_t2": nc.dram_tensor("d_t2", [vpad, 32], F16,
                                   kind="ExternalOutput"),
            "d_wch": nc.dram_tensor("d_wch", [P, CS * 8], F32,
                                    kind="ExternalOutput"),
            "d_wnb": nc.dram_tensor("d_wnb", [P, CS * 4], F32,
                                    kind="ExternalOutput"),
            "d_part": nc.dram_tensor("d_part", [P, 8], F32,
                                     kind="ExternalOutput"),
            "d_mx": nc.dram_tensor("d_mx", [P, CS * 8], F16,
                                   kind="ExternalOutput"),
        }

    import contextlib
    with contextlib.ExitStack() as st:
        sb = lambda name, shape, dtype: st.enter_context(
            nc.sbuf_tensor(name, shape, dtype))
        sem = lambda name: st.enter_context(nc.semaphore(name))

        w_sb = sb("w_sb", [P, tot_g], F32)
        parts = sb("parts_sb", [P, tot_g * 8], F32)
        idx1_sb = [sb(f"idx1_{i}", [P, CS * 8], I16) for i in range(2)]
        idx2_sb = [sb(f"idx2_{i}", [P, CS * 8], I16) for i in range(2)]
        mask_sb = [sb(f"mask_{i}", [P, CS * 16], F16) for i in range(2)]
        rec1 = [sb(f"rec1_{i}", [P, CS * 32], F16) for i in range(2)]
        rec2 = [sb(f"rec2_{i}", [P, CS * 32], F16) for i in range(2)]
        tmpe = [sb(f"tmpe_{i}", [P, CS * 32], F16) for i in range(2)]
        t1x = [sb(f"t1x_{i}", [P, CS * 2], F32) for i in range(2)]
        pv = [sb(f"pv_{i}", [P, CS * 16], F16) for i in range(2)]
        ta = [sb(f"ta_{i}", [P, CS * 8], F16) for i in range(2)]
        tb = [sb(f"tb_{i}", [P, CS * 8], F16) for i in range(2)]
        dmx = [sb(f"dmx_{i}", [P, CS * 16], F16) for i in range(2)]
        dmn = [sb(f"dmn_{i}", [P, CS * 16], F16) for i in range(2)]
        Mx = [sb(f"Mx_{i}", [P, CS * 8], F16) for i in range(2)]
        mn = [sb(f"mn_{i}", [P, CS * 8], F16) for i in range(2)]
        Sx = [sb(f"Sx_{i}", [P, CS * 8], F32) for i in range(2)]
        Sn = [sb(f"Sn_{i}", [P, CS * 8], F32) for i in range(2)]
        wch = [sb(f"wch_{i}", [P, CS * 8], F32) for i in range(2)]
        wnb = [sb(f"wnb_{i}", [P, CS * 4], F32) for i in range(2)]

        s_io = [sem(f"s_io{i}") for i in range(2)]
        s_g1 = [sem(f"s_g1{i}") for i in range(2)]
        s_g2 = [sem(f"s_g2{i}") for i in range(2)]
        s_cons = [sem(f"s_cons{i}") for i in range(2)]
        s_build = sem("s_build")
        s_d1 = sem("s_d1")
        s_act1 = sem("s_act1")
        s_d2 = sem("s_d2")
        s_act2 = sem("s_act2")
        s_fin = sem("s_fin")
        all_sems = (s_io + s_g1 + s_g2 + s_cons
                    + [s_build, s_d1, s_act1, s_d2, s_act2, s_fin])

        nc.gpsimd.load_library(mlp)

        # ---- T2 build ----
        with (
            nc.sbuf_tensor("pos_sb", [P, nt * 16], F32) as pos_sb,
            nc.sbuf_tensor("oh_sb", [P, nt * 32], F32) as oh_sb,
            nc.sbuf_tensor("ohs_sb", [P, nt * 32], F32) as ohs_sb,
            nc.sbuf_tensor("t2img", [P, nt * 32], F16) as t2img,
        ):
            nc.sync.dma_start(
                pos_sb[:].rearrange("p (t f) -> p t f", t=nt),
                posxy.ap().rearrange("(t p) f -> p t f", p=P),
            ).then_inc(s_build, 16)
            nc.sync.dma_start(
                oh_sb[:].rearrange("p (t f) -> p t f", t=nt),
                oh.ap().rearrange("(t p) f -> p t f", p=P),
            ).then_inc(s_build, 16)
            nc.sync.dma_start(w_sb[:], wd.ap()).then_inc(s_build, 16)

            t2r = t2img[:].rearrange("p (t c) -> p t c", t=nt)
            ohr = ohs_sb[:].rearrange("p (t b f) -> p t b f", t=nt, f=4)
            nc.scalar.wait_ge(s_build, 48)
            nc.scalar.activation(out=ohs_sb[:], in_=oh_sb[:], func=ACT.Copy,
                                 scale=GAMMA)
            nc.scalar.activation(out=t2r[:, :, 0:16],
                                 in_=pos_sb[:].rearrange("p (t f) -> p t f", t=nt),
                                 func=ACT.Copy, scale=GAMMA).then_inc(s_d1, 1)
            nc.vector.wait_ge(s_d1, 1)
            nc.vector.tensor_tensor(out=t2r[:, :, 16:24], in0=ohr[:, :, :, 0],
                                    in1=ohr[:, :, :, 2], op=ALU.subtract)
            nc.vector.tensor_tensor(out=t2r[:, :, 24:32], in0=ohr[:, :, :, 1],
                                    in1=ohr[:, :, :, 3], op=ALU.subtract)
            nc.vector.memset(parts[:], 0.0).then_inc(s_d1, 1)
            nc.sync.wait_ge(s_d1, 2)
            nc.sync.dma_start(
                t2.ap().rearrange("(t p) c -> p t c", p=P)[:, :, 0:32], t2r
            ).then_inc(s_build, 16)

        t1_in = t1.ap()[T1_BASE:, 0:32]
        t2_in = t2.ap()[:, 0:32]

        # ---- chunk pipeline ----
        # Software-pipelined: chunk c's seg2/seg3 (and ACT ln) are emitted
        # after chunk c+1's seg1/exp so the DVE never stalls on the ACT
        # round-trips. All semaphore targets are by chunk index.
        ng = [0, 0]   # cumulative gather count per parity slot
        plan = chunk_plan if mode != "build" else []

        def emit_head(c, k, g, slot_off, g_off):
            cs = g * k
            b = c % 2
            if c >= 2:
                nc.sync.wait_ge(s_cons[b], c // 2)
            nc.sync.dma_start(
                idx1_sb[b][:, 0:cs * 8],
                idx1d.ap()[:, slot_off * 8:(slot_off + cs) * 8],
            ).then_inc(s_io[b], 16)
            nc.sync.dma_start(
                idx2_sb[b][:, 0:cs * 8],
                idx2d.ap()[:, slot_off * 8:(slot_off + cs) * 8],
            ).then_inc(s_io[b], 16)
            nc.sync.dma_start(
                mask_sb[b][:, 0:cs * 16],
                maskd.ap()[:, slot_off * 16:(slot_off + cs) * 16],
            ).then_inc(s_io[b], 16)

            nc.gpsimd.wait_ge(s_io[b], 48 * (c // 2 + 1))
            if c == 0:
                nc.gpsimd.wait_ge(s_build, 64)
            if c >= 2:
                nc.gpsimd.wait_ge(s_cons[b], c // 2)
            r1 = rec1[b][:].rearrange("p (n e) -> p n e", e=32)
            r2 = rec2[b][:].rearrange("p (n e) -> p n e", e=32)
            for s0 in range(0, cs, SUB_COLS):
                s1 = min(s0 + SUB_COLS, cs)
                nidx = (s1 - s0) * P
                dma_gather_raw(nc.gpsimd, r1[:, s0:s1, :], t1_in,
                               idx1_sb[b][:, s0 * 8:s1 * 8], nidx, 32, P,
                               queue_num=(2 * c) % 4).then_inc(s_g1[b], 16)
                dma_gather_raw(nc.gpsimd, r2[:, s0:s1, :], t2_in,
                               idx2_sb[b][:, s0 * 8:s1 * 8], nidx, 32, P,
                               queue_num=(2 * c + 1) % 4).then_inc(s_g2[b], 16)
                ng[b] += 1

            if mode == "gather":
                nc.vector.wait_ge(s_g1[b], 16 * ng[b])
                nc.vector.wait_ge(s_g2[b], 16 * ng[b])
                nc.vector.sem_inc(s_cons[b], 1)
                nc.vector.sem_inc(s_fin, 1)
                return

            # DVE seg1: extract + rotate + max/min + shifts
            nc.vector.wait_ge(s_g1[b], 16 * ng[b])
            nc.vector.wait_ge(s_io[b], 48 * (c // 2 + 1))
            r1v = rec1[b][:].rearrange("p (n j two) -> p n two j", two=2,
                                       j=PINS_PER_ROW)[:, 0:cs, :, :]
            mkb = (mask_sb[b][:].rearrange("p (n j) -> p n j", j=16)
                   [:, 0:cs, :].unsqueeze(2).to_broadcast([P, cs, 2, 16]))
            tev = tmpe[b][:].rearrange("p (n t j) -> p n t j", t=2, j=16)[:, 0:cs]
            nc.vector.tensor_tensor(out=tev, in0=r1v, in1=mkb, op=ALU.mult)
            nc.vector.tensor_reduce(
                out=t1x[b][:, 0:cs * 2],
                in_=tmpe[b][:].rearrange("p (s j) -> p s j", j=16)[:, 0:cs * 2],
                axis=AX.X, op=ALU.add)

            nc.vector.wait_ge(s_g2[b], 16 * ng[b])
            r5 = r2[:, 0:cs, :].rearrange("p (g j) c -> p g j c", g=g)
            Xv = r5[:, :, :, 0:8]
            Yv = r5[:, :, :, 8:16]
            Cv = r5[:, :, :, 16:24]
            Sv = r5[:, :, :, 24:32]
            t1r = t1x[b][:, 0:cs * 2].rearrange("p (g j two) -> p g j two",
                                                g=g, two=2)
            oxv = t1r[:, :, :, 0:1].to_broadcast([P, g, k, 8])
            oyv = t1r[:, :, :, 1:2].to_broadcast([P, g, k, 8])
            pvr = pv[b][:, 0:cs * 16].rearrange("p (g b c j) -> p g b c j",
                                                g=g, b=8, c=2)
            pxo = pvr[:, :, :, 0, :].transpose([0, 1, 3, 2])
            pyo = pvr[:, :, :, 1, :].transpose([0, 1, 3, 2])
            tar = ta[b][:, 0:cs * 8].rearrange("p (g j b) -> p g j b", g=g, b=8)
            tbr = tb[b][:, 0:cs * 8].rearrange("p (g j b) -> p g j b", g=g, b=8)

            nc.vector.tensor_tensor(out=tar, in0=Cv, in1=oxv, op=ALU.mult)
            nc.vector.tensor_tensor(out=tbr, in0=Sv, in1=oyv, op=ALU.mult)
            nc.vector.tensor_tensor(out=tar, in0=tar, in1=Xv, op=ALU.add)
            nc.vector.tensor_tensor(out=pxo, in0=tar, in1=tbr, op=ALU.subtract)
            nc.vector.tensor_tensor(out=tar, in0=Sv, in1=oxv, op=ALU.mult)
            nc.vector.tensor_tensor(out=tbr, in0=Cv, in1=oyv, op=ALU.mult)
            nc.vector.tensor_tensor(out=tar, in0=tar, in1=Yv, op=ALU.add)
            nc.vector.tensor_tensor(out=pyo, in0=tar, in1=tbr,
                                    op=ALU.add).then_inc(s_cons[b], 1)

            nn = g * 16
            pvs = pv[b][:, 0:cs * 16].rearrange("p (s j) -> p s j", j=k)
            nc.vector.tensor_reduce(out=Mx[b][:, 0:nn], in_=pvs, axis=AX.X,
                                    op=ALU.max)
            nc.vector.tensor_reduce(out=mn[b][:, 0:nn], in_=pvs, axis=AX.X,
                                    op=ALU.min)
            Mb = Mx[b][:, 0:nn].unsqueeze(2).to_broadcast([P, nn, k])
            mb = mn[b][:, 0:nn].unsqueeze(2).to_broadcast([P, nn, k])
            dxr = dmx[b][:, 0:cs * 16].rearrange("p (s j) -> p s j", j=k)
            dnr = dmn[b][:, 0:cs * 16].rearrange("p (s j) -> p s j", j=k)
            nc.vector.tensor_tensor(out=dxr, in0=pvs, in1=Mb, op=ALU.subtract)
            nc.vector.tensor_tensor(out=dnr, in0=pvs, in1=mb,
                                    op=ALU.subtract).then_inc(s_d1, 1)

            if mode == "seg1":
                nc.vector.sem_inc(s_fin, 1)
                return
            # ACT: exps (in place)
            nc.scalar.wait_ge(s_d1, 3 + c)
            nc.scalar.activation(out=dmx[b][:, 0:cs * 16],
                                 in_=dmx[b][:, 0:cs * 16], func=ACT.Exp)
            nc.scalar.activation(out=dmn[b][:, 0:cs * 16],
                                 in_=dmn[b][:, 0:cs * 16], func=ACT.Exp,
                                 scale=-1.0).then_inc(s_act1, 1)

        def emit_tail(c, k, g, slot_off, g_off):
            cs = g * k
            b = c % 2
            nn = g * 16
            dxr = dmx[b][:, 0:cs * 16].rearrange("p (s j) -> p s j", j=k)
            dnr = dmn[b][:, 0:cs * 16].rearrange("p (s j) -> p s j", j=k)
            # DVE seg2: sums + extent
            nc.vector.wait_ge(s_act1, c + 1)
            nc.vector.tensor_reduce(out=Sx[b][:, 0:nn], in_=dxr, axis=AX.X,
                                    op=ALU.add)
            nc.vector.tensor_reduce(out=Sn[b][:, 0:nn], in_=dnr, axis=AX.X,
                                    op=ALU.add)
            nc.vector.tensor_tensor(out=wch[b][:, 0:nn], in0=Mx[b][:, 0:nn],
                                    in1=mn[b][:, 0:nn],
                                    op=ALU.subtract).then_inc(s_d2, 1)
            # ACT: logs (in place)
            nc.scalar.wait_ge(s_d2, c + 1)
            nc.scalar.activation(out=Sx[b][:, 0:nn], in_=Sx[b][:, 0:nn],
                                 func=ACT.Ln)
            nc.scalar.activation(out=Sn[b][:, 0:nn], in_=Sn[b][:, 0:nn],
                                 func=ACT.Ln).then_inc(s_act2, 1)
            # DVE seg3: combine + weight into parts slice
            nc.vector.wait_ge(s_act2, c + 1)
            nc.vector.tensor_tensor(out=wch[b][:, 0:nn], in0=wch[b][:, 0:nn],
                                    in1=Sx[b][:, 0:nn], op=ALU.add)
            nc.vector.tensor_tensor(out=wch[b][:, 0:nn], in0=wch[b][:, 0:nn],
                                    in1=Sn[b][:, 0:nn], op=ALU.add)
            pslice = parts[:, g_off * 8:(g_off + g) * 8]
            nc.vector.tensor_reduce(
                out=pslice,
                in_=wch[b][:, 0:nn].rearrange("p (s c) -> p s c", c=2),
                axis=AX.X, op=ALU.add)
            wbr = (w_sb[:, g_off:g_off + g].unsqueeze(2)
                   .to_broadcast([P, g, 8]))
            psv = pslice.rearrange("p (g b) -> p g b", g=g)
            nc.vector.tensor_tensor(out=psv, in0=psv, in1=wbr,
                                    op=ALU.mult).then_inc(s_fin, 1)

        prev = None
        for c, ck in enumerate(plan):
            emit_head(c, *ck)
            if mode in ("gather", "seg1"):
                continue
            if prev is not None:
                emit_tail(prev[0], *prev[1])
            prev = (c, ck)
        if prev is not None and mode not in ("gather", "seg1"):
            emit_tail(prev[0], *prev[1])

        # ---- writeback + cleanup ----
        if mode != "build":
            nc.sync.wait_ge(s_fin, nch)
        nfin = 80
        if dump is not None:
            dp = dump_ci % 2
            nc.sync.dma_start(dump["d_rec1"].ap(), rec1[dp][:]).then_inc(s_build, 16)
            nc.sync.dma_start(dump["d_rec2"].ap(), rec2[dp][:]).then_inc(s_build, 16)
            nc.sync.dma_start(dump["d_t1x"].ap(), t1x[dp][:]).then_inc(s_build, 16)
            nc.sync.dma_start(dump["d_pv"].ap(), pv[dp][:]).then_inc(s_build, 16)
            nc.sync.dma_start(dump["d_dmx"].ap(), dmx[dp][:]).then_inc(s_build, 16)
            nc.sync.dma_start(dump["d_dmn"].ap(), dmn[dp][:]).then_inc(s_build, 16)
            nc.sync.dma_start(dump["d_sx"].ap(), Sx[dp][:]).then_inc(s_build, 16)
            nc.sync.dma_start(dump["d_sn"].ap(), Sn[dp][:]).then_inc(s_build, 16)
            nc.sync.dma_start(dump["d_t2"].ap(),
                              t2.ap()[:, 0:32]).then_inc(s_build, 16)
            nc.sync.dma_start(dump["d_wch"].ap(), wch[dp][:]).then_inc(s_build, 16)
            nc.sync.dma_start(dump["d_wnb"].ap(), wnb[dp][:]).then_inc(s_build, 16)
            nc.sync.dma_start(dump["d_part"].ap(),
                              parts[:, 0:8]).then_inc(s_build, 16)
            nc.sync.dma_start(dump["d_mx"].ap(), Mx[dp][:]).then_inc(s_build, 16)
            nfin = 288
        nc.sync.dma_start(out.ap(), parts[:]).then_inc(s_build, 16)
        nc.sync.wait_ge(s_build, nfin)
        nc.all_engine_barrier()
        for s in all_sems:
            nc.gpsimd.sem_clear(s)
        nc.all_engine_barrier()

    lower_extended_insts(nc)
    return nc


def prep_host(positions, pin_offsets, rotation_onehot, net_weights,
              net_to_pin, pin_to_macro):
    """Host-side sharding + metadata layout. Returns (meta, in_maps)."""
    B, V, _ = positions.shape
    Pn = pin_offsets.shape[0]
    N, M = net_to_pin.shape

    vpad = ((V + 1 + P - 1) // P) * P
    pad_mac = V
    n_t1_rows = (Pn + PINS_PER_ROW - 1) // PINS_PER_ROW

    n2p = net_to_pin.astype(np.int32)
    p2m = pin_to_macro.astype(np.int32)

    t1 = np.zeros((n_t1_rows, P), np.float16)
    t1[:, 0:32] = pin_offsets.astype(np.float16).reshape(n_t1_rows, 32)
    posxy = np.zeros((vpad, 16), np.float32)
    posxy[:V, 0:8] = positions[:, :, 0].T
    posxy[:V, 8:16] = positions[:, :, 1].T
    oh = np.zeros((vpad, 32), np.float32)
    oh[:V] = rotation_onehot.transpose(1, 0, 2).reshape(V, 4 * B)

    lengths = (n2p >= 0).sum(axis=1)

    per = (N + N_CORES - 1) // N_CORES
    shards = [(c * per, min((c + 1) * per, N)) for c in range(N_CORES)]

    counts = np.zeros((N_CORES, M + 1), np.int64)
    for c, (a, b) in enumerate(shards):
        counts[c] = np.bincount(lengths[a:b], minlength=M + 1)
    gk = {k: int(-(-counts[:, k].max() // P))
          for k in range(1, M + 1) if counts[:, k].max() > 0}

    chunk_plan = []
    slot_off = 0
    g_off = 0
    bucket_offs = {}
    for k in sorted(gk):
        g_total = gk[k]
        gmax = max(1, MAX_COLS // k)
        bucket_offs[k] = (slot_off, g_off)
        nsplit = -(-g_total // gmax)
        g_per = -(-g_total // nsplit)  # balanced split, no tiny tails
        g_done = 0
        while g_done < g_total:
            g = min(g_per, g_total - g_done)
            chunk_plan.append((k, g, slot_off, g_off))
            slot_off += g * k
            g_off += g
            g_done += g
    tot_slot = slot_off
    tot_g = g_off

    # global columns where a pin sub-gather ends: last column of each
    # SUB_COLS block within each chunk (plus the chunk tail)
    boundary_cols = set()
    for (k, g, so, go) in chunk_plan:
        cs = g * k
        for s0 in range(0, cs, SUB_COLS):
            boundary_cols.add(so + min(s0 + SUB_COLS, cs) - 1)

    def wrap16(vals):
        L = vals.shape[0] // 16
        w = vals.reshape(L, 16).T.astype(np.int16)
        return np.tile(w, (8, 1))

    in_maps = []
    rng = np.random.default_rng(12345)
    for c, (a, b) in enumerate(shards):
        pin_t = np.full((P, tot_slot), HIGH_PIN, np.int32)
        mac_t = np.full((P, tot_slot), pad_mac, np.int32)
        w_all = np.zeros((P, tot_g), np.float32)
        ln = lengths[a:b]
        for k in sorted(gk):
            so, go = bucket_offs[k]
            sel = np.nonzero(ln == k)[0]
            nk = len(sel)
            gkk = gk[k]
            ids = np.full((gkk * P, k), HIGH_PIN, np.int32)
            wp_ = np.zeros((gkk * P,), np.float32)
            if nk:
                ids[:nk] = n2p[a:b][sel][:, :k]
                wp_[:nk] = net_weights[a:b][sel].astype(np.float32) / GAMMA

            # fix sub-gather boundary positions: net at (g_loc, p=127) must
            # have pins >= HIGH_PIN at every boundary j of its group
            for g_loc in range(gkk):
                js = [(col - so) % k for col in range(so + g_loc * k,
                                                      so + (g_loc + 1) * k)
                      if col in boundary_cols]
                if not js:
                    continue
                row127 = g_loc * P + 127
                lo = g_loc * P
                hi = min(g_loc * P + P, gkk * P)
                cand_rows = None
                cur = ids[row127] if row127 < gkk * P else None
                if cur is not None and (cur >= HIGH_PIN).sum() >= len(js):
                    cand_rows = row127
                else:
                    high_counts = (ids[lo:hi] >= HIGH_PIN).sum(axis=1)
                    ok_rows = np.nonzero(high_counts >= len(js))[0]
                    assert len(ok_rows) > 0, (
                        f"no boundary-safe net in bucket k={k} group {g_loc}")
                    cand_rows = lo + ok_rows[0]
                if cand_rows != row127:
                    ids[[row127, cand_rows]] = ids[[cand_rows, row127]]
                    wp_[[row127, cand_rows]] = wp_[[cand_rows, row127]]
                # reorder pins within the row127 net: place high pins at js
                row = ids[row127].copy()
                high = np.nonzero(row >= HIGH_PIN)[0]
                rest = [i for i in range(k) if i not in set(high[:len(js)])]
                newrow = np.empty(k, np.int32)
                used = set()
                for j, hsrc in zip(js, high):
                    newrow[j] = row[hsrc]
                    used.add(hsrc)
                fill = [row[i] for i in range(k) if i not in used]
                fi = 0
                for j in range(k):
                    if j not in set(js):
                        newrow[j] = fill[fi]
                        fi += 1
                ids[row127] = newrow

            pin_t[:, so:so + gkk * k] = (
                ids.reshape(gkk, P, k).transpose(1, 0, 2).reshape(P, gkk * k))
            w_all[:, go:go + gkk] = wp_.reshape(gkk, P).T

        valid_t = pin_t < Pn
        mac_t = np.where(valid_t, p2m[np.where(valid_t, pin_t, 0)], pad_mac)

        idx1 = np.zeros((P, tot_slot * 8), np.int16)
        idx2 = np.zeros((P, tot_slot * 8), np.int16)
        for (k, g, so, go) in chunk_plan:
            cs = g * k
            blk_pin = pin_t[:, so:so + cs].T.reshape(-1)
            blk_mac = mac_t[:, so:so + cs].T.reshape(-1)
            idx1[:, so * 8:(so + cs) * 8] = wrap16(
                blk_pin // PINS_PER_ROW - T1_BASE)
            idx2[:, so * 8:(so + cs) * 8] = wrap16(blk_mac)

        maskt = (np.arange(16)[None, None, :]
                 == (pin_t % PINS_PER_ROW)[:, :, None]).astype(np.float16)
        in_maps.append({
            "t1": t1, "posxy": posxy, "oh": oh,
            "idx1": idx1, "idx2": idx2,
            "maskt": maskt.reshape(P, tot_slot * 16),
            "w_all": w_all,
        })

    meta = (vpad, n_t1_rows, tuple(chunk_plan), tot_slot, tot_g)
    return meta, in_maps


_prog_cache = {}


def kernel(**inputs):
    meta, in_maps = prep_host(
        np.asarray(inputs["positions"]),
        np.asarray(inputs["pin_offsets"]),
        np.asarray(inputs["rotation_onehot"]),
        np.asarray(inputs["net_weights"]),
        np.asarray(inputs["net_to_pin"]),
        np.asarray(inputs["pin_to_macro"]),
    )
    if meta not in _prog_cache:
        _prog_cache[meta] = build_program(*meta)
    nc = _prog_cache[meta]
    res = bass_utils.run_bass_kernel_spmd(nc, in_maps, core_ids=list(range(N_CORES)))
    total = np.zeros(8, np.float64)
    for r in res.results:
        a = r["acc"].astype(np.float64)
        total += a.reshape(a.shape[0], -1, 8).sum(axis=(0, 1))
    return total.astype(np.float32)
